# revision 20
# baseline (speedup 1.0000x reference)
import os
import sys

for _p in ("/opt/trn_rl_repo", "/root/.axon_site/_ro/trn_rl_repo"):
    if os.path.isdir(_p) and _p not in sys.path:
        sys.path.insert(0, _p)

import numpy as np

import concourse.bass as bass
import concourse.bacc as bacc
import concourse.mybir as mybir
import concourse.tile as tile
from concourse import masks
from concourse.bass_utils import run_bass_kernel_spmd

f32 = mybir.dt.float32
f32r = mybir.dt.float32r
AF = mybir.ActivationFunctionType
ALU = mybir.AluOpType

D = 72
EPS = 1e-6

TRACE = False
LAST_RESULT = None


def _windows(total, w):
    return [(s, min(w, total - s)) for s in range(0, total, w)]


def build_nc(B, L1, L2, M, H, NC):
    """Cross-modality bi-attention block, query(L1)-axis sharded over NC cores.

    Feature-major ("T") tensors are [M, tokens], token index = b*Lb + t
    (b-major). Softmaxes skip max-subtraction (logits are ~N(0,1)); softmax
    sums come from a ones-column appended to the v blocks (psum row D). The
    o2/s2 partials (reduced over the sharded q axis) go through one AllReduce;
    out1 is fully local. Matmuls run as float32r (full PE rate at N>=256).
    """
    HID = H * D
    assert M == HID
    LSH = L1 // NC
    T1 = B * LSH
    T2 = B * L2
    SCALE = float(D) ** -0.5

    MT = _windows(M, 128)
    NMT = len(MT)
    HW = _windows(M, 512)
    QT = _windows(LSH, 128)
    KT = _windows(L2, 128)
    T1T = _windows(T1, 128)
    VW = D + 1

    nc = bacc.Bacc("TRN2", target_bir_lowering=False, debug=False,
                   num_devices=NC)

    def din(name, shape):
        return nc.dram_tensor(name, shape, f32, kind="ExternalInput").ap()

    x1n = din("x1n", [T1, M])
    x2n = din("x2n", [T2, M])
    Wq = din("Wq", [M, M])
    Wk = din("Wk", [M, M])
    Wv1 = din("Wv1", [M, M])
    Wv2 = din("Wv2", [M, M])
    Wo1 = din("Wo1", [M, M])
    Wo2 = din("Wo2", [M, M])
    bqh = din("bqh", [D, H])
    bkh = din("bkh", [D, H])
    bv1r = din("bv1r", [1, M])
    bv2r = din("bv2r", [1, M])
    bo1r = din("bo1r", [1, M])
    bo2r = din("bo2r", [1, M])
    g1 = din("g1", [1, M])
    g2 = din("g2", [1, M])
    ones_d = din("ones_d", [128, 128])
    out1 = nc.dram_tensor("out1", [T1, M], f32, kind="ExternalOutput").ap()
    out2 = nc.dram_tensor("out2", [T2, M], f32, kind="ExternalOutput").ap()

    with tile.TileContext(nc) as tc:
        with (
            tc.tile_pool(name="res", bufs=1) as res,
            tc.tile_pool(name="psA", bufs=4, space="PSUM") as psA,
            tc.tile_pool(name="psB", bufs=4, space="PSUM") as psB,
            tc.tile_pool(name="dram", bufs=1, space="DRAM") as dram,
        ):
            o1P_d = dram.tile([M, T1], f32, tag="o1P_d")
            red_in = dram.tile([B, M + H, L2], f32, tag="red_in")
            red_out = dram.tile([B, M + H, L2], f32, tag="red_out",
                                addr_space="Shared" if NC > 4 else "Local")
            qT_d = dram.tile([H, D, T1], f32, tag="qT_d")
            kT_d = dram.tile([H, D, T2], f32, tag="kT_d")
            o2n_d = dram.tile([B, M, L2], f32, tag="o2n_d")

            ones_row = res.tile([1, 128], f32, tag="ones_row")
            nc.sync.dma_start(ones_row[:].bitcast(f32r),
                              ones_d[0:1, :].bitcast(f32r))
            eps_col = res.tile([128, 1], f32, tag="eps_col")
            nc.vector.memset(eps_col[:], EPS)
            ident_g = res.tile([128, 128], f32, tag="ident_g")
            masks.make_identity(nc, ident_g[:])
            # DVE-owned copy: transpose matmuls may carry only ONE sync wait,
            # so every transpose operand must be produced on DVE
            ident = res.tile([128, 128], f32, tag="ident")
            nc.vector.tensor_copy(ident[:], ident_g[:])

            def load_row(dr, shape, tag, rnd=False):
                t = res.tile(shape, f32, tag=tag)
                if rnd:
                    nc.sync.dma_start(t[:].bitcast(f32r), dr[:].bitcast(f32r))
                else:
                    nc.sync.dma_start(t[:], dr[:])
                return t

            bq_sb = load_row(bqh, [D, H], "bq_sb")
            bk_sb = load_row(bkh, [D, H], "bk_sb")
            bv1_sb = load_row(bv1r, [1, M], "bv1_sb", rnd=True)
            bv2_sb = load_row(bv2r, [1, M], "bv2_sb", rnd=True)
            bo1_sb = load_row(bo1r, [1, M], "bo1_sb", rnd=True)
            bo2_sb = load_row(bo2r, [1, M], "bo2_sb", rnd=True)
            g1_sb = load_row(g1, [1, M], "g1_sb", rnd=True)
            g2_sb = load_row(g2, [1, M], "g2_sb", rnd=True)

            with tc.tile_pool(name="vp", bufs=1) as vp:
                with tc.tile_pool(name="xmp", bufs=1) as xmp:
                    xm1 = [xmp.tile([128, T1], f32, tag=f"x1_{m0}",
                                    name=f"xm1_{m0}")
                           for (m0, mm) in MT]
                    xm2 = [xmp.tile([128, T2], f32, tag=f"x2_{m0}",
                                    name=f"xm2_{m0}")
                           for (m0, mm) in MT]

                    # ---- LayerNorm in natural layout + PE transpose -------
                    with tc.tile_pool(name="lnp", bufs=2) as lnp:
                        def layernorm(x_dram, T, xmT):
                            for (p0, pn) in _windows(T, 128):
                                xt = lnp.tile([128, M], f32, tag="xnat")
                                nc.sync.dma_start(xt[:pn, :],
                                                  x_dram[p0:p0 + pn, :])
                                sq = lnp.tile([128, M], f32, tag="sq")
                                nc.scalar.activation(sq[:pn, :], xt[:pn, :],
                                                     AF.Square)
                                sx = lnp.tile([128, 1], f32, tag="sx")
                                nc.vector.tensor_reduce(
                                    sx[:pn, :], xt[:pn, :],
                                    axis=mybir.AxisListType.X, op=ALU.add)
                                sx2 = lnp.tile([128, 1], f32, tag="sx2")
                                nc.vector.tensor_reduce(
                                    sx2[:pn, :], sq[:pn, :],
                                    axis=mybir.AxisListType.X, op=ALU.add)
                                mu = lnp.tile([128, 1], f32, tag="mu")
                                nc.vector.tensor_scalar_mul(
                                    mu[:pn, :], sx[:pn, :], 1.0 / M)
                                var = lnp.tile([128, 1], f32, tag="var")
                                nc.vector.tensor_scalar_mul(
                                    var[:pn, :], sx2[:pn, :], 1.0 / M)
                                tmp = lnp.tile([128, 1], f32, tag="tmp")
                                nc.vector.tensor_mul(tmp[:pn, :], mu[:pn, :],
                                                     mu[:pn, :])
                                nc.vector.tensor_sub(var[:pn, :], var[:pn, :],
                                                     tmp[:pn, :])
                                std = lnp.tile([128, 1], f32, tag="std")
                                nc.scalar.activation(std[:pn, :], var[:pn, :],
                                                     AF.Sqrt,
                                                     bias=eps_col[:pn, :])
                                rstd = lnp.tile([128, 1], f32, tag="rstd")
                                nc.vector.reciprocal(rstd[:pn, :], std[:pn, :])
                                xmn = lnp.tile([128, M], f32, tag="xmn")
                                nc.vector.tensor_scalar(
                                    xmn[:pn, :], xt[:pn, :],
                                    mu[:pn, :], rstd[:pn, :],
                                    op0=ALU.subtract, op1=ALU.mult)
                                for i, (m0, mm) in enumerate(MT):
                                    tp = psA.tile([128, 128], f32, tag="A")
                                    nc.tensor.transpose(
                                        tp[:mm, :pn], xmn[:pn, m0:m0 + mm],
                                        ident[:pn, :pn])
                                    with nc.allow_low_precision(
                                            reason="feeds fp32r matmul"):
                                        nc.vector.tensor_copy(
                                            xmT[i][:mm, p0:p0 + pn]
                                            .bitcast(f32r), tp[:mm, :pn])

                        layernorm(x1n, T1, xm1)
                        layernorm(x2n, T2, xm2)

                    # ---- q/k projections, all heads -> DRAM ---------------
                    with (
                        tc.tile_pool(name="wqk", bufs=NMT + 1) as pwqk,
                        tc.tile_pool(name="qks", bufs=3) as pqks,
                    ):
                        def qk_proj(xm, W, b_sb, T, scale, dst, h):
                            stage = pqks.tile([D, max(T1, T2)], f32,
                                              tag="qks")
                            wts = []
                            for i, (m0, mm) in enumerate(MT):
                                wt = pwqk.tile([128, D], f32, tag="wqk")
                                nc.sync.dma_start(
                                    wt[:mm, :].bitcast(f32r),
                                    W[m0:m0 + mm, h * D:(h + 1) * D]
                                    .bitcast(f32r))
                                wts.append(wt)
                            for (t0, tn) in _windows(T, 512):
                                ps = psA.tile([128, 512], f32, tag="A")
                                for i, (m0, mm) in enumerate(MT):
                                    nc.tensor.matmul(
                                        ps[:D, :tn],
                                        wts[i][:mm, :].bitcast(f32r),
                                        xm[i][:mm, t0:t0 + tn].bitcast(f32r),
                                        start=(i == 0), stop=(i == NMT - 1))
                                nc.vector.tensor_scalar(
                                    stage[:, t0:t0 + tn], ps[:D, :tn],
                                    scale, b_sb[:, h:h + 1],
                                    op0=ALU.mult, op1=ALU.add)
                            nc.sync.dma_start(dst[h, :, :], stage[:, :T])

                        for h in range(H):
                            qk_proj(xm1, Wq, bq_sb, T1, SCALE, qT_d, h)
                            qk_proj(xm2, Wk, bk_sb, T2, 1.0, kT_d, h)

                    # ---- v projections (per-head 73-blocks + ones col) ----
                    with tc.tile_pool(name="wvp", bufs=NMT + 1) as pwv:
                        def v_proj(xm, Wv, bv_sb, T, tag):
                            tt_list = _windows(T, 128)
                            ve = []
                            for ti, (p0, pn) in enumerate(tt_list):
                                vt = vp.tile([128, H * VW], f32,
                                             tag=f"v{tag}_{p0}")
                                nc.sync.dma_start(
                                    vt.rearrange("p (h c) -> p h c",
                                                 c=VW)[:, :, D]
                                    .bitcast(f32r),
                                    ones_d[:, :H].bitcast(f32r))
                                ve.append(vt)
                            for (n0, nn) in HW:
                                wvs = []
                                for i, (m0, mm) in enumerate(MT):
                                    wv = pwv.tile([128, 512], f32, tag="wv")
                                    nc.sync.dma_start(
                                        wv[:mm, :nn].bitcast(f32r),
                                        Wv[m0:m0 + mm, n0:n0 + nn]
                                        .bitcast(f32r))
                                    wvs.append(wv)
                                for ti, (p0, pn) in enumerate(tt_list):
                                    ps = psA.tile([128, 512], f32, tag="A")
                                    for i, (m0, mm) in enumerate(MT):
                                        nc.tensor.matmul(
                                            ps[:pn, :nn],
                                            xm[i][:mm, p0:p0 + pn]
                                            .bitcast(f32r),
                                            wvs[i][:mm, :nn].bitcast(f32r),
                                            start=(i == 0), stop=False)
                                    nc.tensor.matmul(
                                        ps[:pn, :nn],
                                        ones_row[:, :pn].bitcast(f32r),
                                        bv_sb[:, n0:n0 + nn].bitcast(f32r),
                                        start=False, stop=True)
                                    h0 = n0 // D
                                    h1 = (n0 + nn - 1) // D
                                    for h in range(h0, h1 + 1):
                                        c0 = max(n0, h * D)
                                        c1 = min(n0 + nn, (h + 1) * D)
                                        nc.scalar.activation(
                                            ve[ti][:pn,
                                                   h * VW + (c0 - h * D):
                                                   h * VW + (c1 - h * D)]
                                            .bitcast(f32r),
                                            ps[:pn, c0 - n0:c1 - n0],
                                            AF.Copy)
                            return ve

                        v1e = v_proj(xm1, Wv1, bv1_sb, T1, "1")
                        v2e = v_proj(xm2, Wv2, bv2_sb, T2, "2")
                # xmp closed: xm freed

                # ---- bi-attention per (h, b) ---------------------------
                with (
                    tc.tile_pool(name="pe", bufs=8) as pe,
                    tc.tile_pool(name="qkl", bufs=3) as pqkl,
                    tc.tile_pool(name="stB", bufs=2) as pstB,
                ):
                    for h in range(H):
                        for b in range(B):
                            qh = pqkl.tile([D, 512], f32, tag="qh")
                            nc.sync.dma_start(
                                qh[:, :LSH].bitcast(f32r),
                                qT_d[h, :, b * LSH:(b + 1) * LSH]
                                .bitcast(f32r))
                            kh = pqkl.tile([D, 512], f32, tag="kh")
                            nc.sync.dma_start(
                                kh[:, :L2].bitcast(f32r),
                                kT_d[h, :, b * L2:(b + 1) * L2].bitcast(f32r))
                            # logits transposed [k, q] -> e1
                            e1 = []
                            for (k0, kn) in KT:
                                lps = psA.tile([128, 512], f32, tag="A")
                                nc.tensor.matmul(
                                    lps[:kn, :LSH],
                                    kh[:, k0:k0 + kn].bitcast(f32r),
                                    qh[:, :LSH].bitcast(f32r),
                                    start=True, stop=True)
                                e = pe.tile([128, 512], f32, tag="e")
                                nc.scalar.activation(e[:kn, :LSH]
                                                     .bitcast(f32r),
                                                     lps[:kn, :LSH], AF.Exp)
                                e1.append(e)
                            # o1T = [v2e_b | 1].T @ e1 -> [VW, LSH]
                            o1ps = psB.tile([VW, 512], f32, tag="B")
                            for ki, (k0, kn) in enumerate(KT):
                                vti = (b * L2 + k0) // 128
                                nc.tensor.matmul(
                                    o1ps[:, :LSH],
                                    v2e[vti][:kn, h * VW:(h + 1) * VW]
                                    .bitcast(f32r),
                                    e1[ki][:kn, :LSH].bitcast(f32r),
                                    start=(ki == 0),
                                    stop=(ki == len(KT) - 1))
                            o1r = pstB.tile([VW, 512], f32, tag="o1r")
                            nc.scalar.activation(o1r[:, :LSH],
                                                 o1ps[:, :LSH], AF.Copy)
                            s1r = pstB.tile([1, 512], f32, tag="s1r")
                            nc.sync.dma_start(s1r[:, :LSH],
                                              o1r[D:D + 1, :LSH])
                            rs1 = pstB.tile([1, 512], f32, tag="rs1")
                            with nc.allow_low_precision(
                                    reason="feeds fp32r matmul"):
                                nc.vector.reciprocal(
                                    rs1[:, :LSH].bitcast(f32r),
                                    s1r[:, :LSH])
                            rb = psA.tile([128, 512], f32, tag="A")
                            nc.tensor.matmul(
                                rb[:D, :LSH], ones_row[:, :D].bitcast(f32r),
                                rs1[:, :LSH].bitcast(f32r),
                                start=True, stop=True)
                            o1n = pstB.tile([D, 512], f32, tag="o1n")
                            nc.vector.tensor_mul(o1n[:, :LSH], o1r[:D, :LSH],
                                                 rb[:D, :LSH])
                            nc.sync.dma_start(
                                o1P_d[h * D:(h + 1) * D,
                                      b * LSH:(b + 1) * LSH],
                                o1n[:, :LSH])
                            # logits natural [q, k] -> e2
                            e2 = []
                            for (q0, qn) in QT:
                                lps = psA.tile([128, 512], f32, tag="A")
                                nc.tensor.matmul(
                                    lps[:qn, :L2],
                                    qh[:, q0:q0 + qn].bitcast(f32r),
                                    kh[:, :L2].bitcast(f32r),
                                    start=True, stop=True)
                                e = pe.tile([128, 512], f32, tag="e")
                                nc.scalar.activation(e[:qn, :L2]
                                                     .bitcast(f32r),
                                                     lps[:qn, :L2], AF.Exp)
                                e2.append(e)
                            # o2t = [v1e_b | 1].T @ e2 -> [VW, L2]
                            o2ps = psB.tile([VW, 512], f32, tag="B")
                            for qi, (q0, qn) in enumerate(QT):
                                vti = (b * LSH + q0) // 128
                                nc.tensor.matmul(
                                    o2ps[:, :L2],
                                    v1e[vti][:qn, h * VW:(h + 1) * VW]
                                    .bitcast(f32r),
                                    e2[qi][:qn, :L2].bitcast(f32r),
                                    start=(qi == 0),
                                    stop=(qi == len(QT) - 1))
                            o2st = pstB.tile([VW, 512], f32, tag="o2st")
                            nc.scalar.activation(o2st[:, :L2], o2ps[:, :L2],
                                                 AF.Copy)
                            nc.sync.dma_start(
                                red_in[b, h * D:(h + 1) * D, :],
                                o2st[:D, :L2])
                            nc.sync.dma_start(red_in[b, M + h, :],
                                              o2st[D:D + 1, :L2])
            # vp closed

            # ---- AllReduce of o2 partials + s2 -----------------------------
            nc.gpsimd.collective_compute(
                "AllReduce", ALU.add,
                replica_groups=[list(range(NC))],
                ins=[red_in[:]],
                outs=[red_out[:]],
            )

            with (
                tc.tile_pool(name="c1", bufs=1) as c1,
                tc.tile_pool(name="wvc", bufs=NMT + 1) as pwvc,
                tc.tile_pool(name="ioc", bufs=3) as pio,
                tc.tile_pool(name="stC", bufs=2) as pstC,
            ):
                g1b = c1.tile([128, M], f32, tag="g1b")
                g2b = c1.tile([128, M], f32, tag="g2b")
                for (n0, nn) in HW:
                    for g_sb, gb in ((g1_sb, g1b), (g2_sb, g2b)):
                        ps = psA.tile([128, 512], f32, tag="A")
                        nc.tensor.matmul(ps[:, :nn],
                                         ones_row[:].bitcast(f32r),
                                         g_sb[:, n0:n0 + nn].bitcast(f32r),
                                         start=True, stop=True)
                        nc.scalar.activation(gb[:, n0:n0 + nn], ps[:, :nn],
                                             AF.Copy)

                # ---- dx1 / out1 (overlaps the AllReduce) -------------------
                o1Pt = []
                for i, (m0, mm) in enumerate(MT):
                    t = c1.tile([128, T1], f32, tag=f"o1P_{m0}")
                    nc.sync.dma_start(t[:mm, :].bitcast(f32r),
                                      o1P_d[m0:m0 + mm, :].bitcast(f32r))
                    o1Pt.append(t)
                for (n0, nn) in HW:
                    wos = []
                    for i, (m0, mm) in enumerate(MT):
                        wo = pwvc.tile([128, 512], f32, tag="wv")
                        nc.sync.dma_start(wo[:mm, :nn].bitcast(f32r),
                                          Wo1[m0:m0 + mm, n0:n0 + nn]
                                          .bitcast(f32r))
                        wos.append(wo)
                    for (p0, pn) in T1T:
                        ps = psA.tile([128, 512], f32, tag="A")
                        for i, (m0, mm) in enumerate(MT):
                            nc.tensor.matmul(
                                ps[:pn, :nn],
                                o1Pt[i][:mm, p0:p0 + pn].bitcast(f32r),
                                wos[i][:mm, :nn].bitcast(f32r),
                                start=(i == 0), stop=False)
                        nc.tensor.matmul(
                            ps[:pn, :nn], ones_row[:, :pn].bitcast(f32r),
                            bo1_sb[:, n0:n0 + nn].bitcast(f32r),
                            start=False, stop=True)
                        xg = pio.tile([128, 512], f32, tag="xg")
                        nc.sync.dma_start(xg[:pn, :nn],
                                          x1n[p0:p0 + pn, n0:n0 + nn])
                        ot = pio.tile([128, 512], f32, tag="ot")
                        nc.vector.tensor_mul(ot[:pn, :nn], ps[:pn, :nn],
                                             g1b[:pn, n0:n0 + nn])
                        nc.vector.tensor_add(ot[:pn, :nn], ot[:pn, :nn],
                                             xg[:pn, :nn])
                        nc.sync.dma_start(out1[p0:p0 + pn, n0:n0 + nn],
                                          ot[:pn, :nn])

                # ---- o2 normalize ------------------------------------------
                for b in range(B):
                    for h in range(H):
                        s2 = pstC.tile([1, 512], f32, tag="s2")
                        nc.sync.dma_start(s2[:, :L2], red_out[b, M + h, :])
                        rs2 = pstC.tile([1, 512], f32, tag="rs2")
                        with nc.allow_low_precision(
                                reason="feeds fp32r matmul"):
                            nc.vector.reciprocal(rs2[:, :L2].bitcast(f32r),
                                                 s2[:, :L2])
                        o2l = pstC.tile([D, 512], f32, tag="o2l")
                        nc.sync.dma_start(o2l[:, :L2],
                                          red_out[b, h * D:(h + 1) * D, :])
                        rb = psA.tile([128, 512], f32, tag="A")
                        nc.tensor.matmul(
                            rb[:D, :L2], ones_row[:, :D].bitcast(f32r),
                            rs2[:, :L2].bitcast(f32r), start=True, stop=True)
                        o2n = pstC.tile([D, 512], f32, tag="o2n")
                        nc.vector.tensor_mul(o2n[:, :L2], o2l[:, :L2],
                                             rb[:D, :L2])
                        nc.sync.dma_start(o2n_d[b, h * D:(h + 1) * D, :],
                                          o2n[:, :L2])

                # ---- dx2 / out2 --------------------------------------------
                for b in range(B):
                    o2Pt = []
                    for i, (m0, mm) in enumerate(MT):
                        t = c1.tile([128, 512], f32, tag=f"o2P_{m0}")
                        nc.sync.dma_start(t[:mm, :L2].bitcast(f32r),
                                          o2n_d[b, m0:m0 + mm, :]
                                          .bitcast(f32r))
                        o2Pt.append(t)
                    for (n0, nn) in HW:
                        wos = []
                        for i, (m0, mm) in enumerate(MT):
                            wo = pwvc.tile([128, 512], f32, tag="wv")
                            nc.sync.dma_start(wo[:mm, :nn].bitcast(f32r),
                                              Wo2[m0:m0 + mm, n0:n0 + nn]
                                              .bitcast(f32r))
                            wos.append(wo)
                        for (k0, kn) in KT:
                            ps = psA.tile([128, 512], f32, tag="A")
                            for i, (m0, mm) in enumerate(MT):
                                nc.tensor.matmul(
                                    ps[:kn, :nn],
                                    o2Pt[i][:mm, k0:k0 + kn].bitcast(f32r),
                                    wos[i][:mm, :nn].bitcast(f32r),
                                    start=(i == 0), stop=False)
                            nc.tensor.matmul(
                                ps[:kn, :nn], ones_row[:, :kn].bitcast(f32r),
                                bo2_sb[:, n0:n0 + nn].bitcast(f32r),
                                start=False, stop=True)
                            r0 = b * L2 + k0
                            xg = pio.tile([128, 512], f32, tag="xg")
                            nc.sync.dma_start(xg[:kn, :nn],
                                              x2n[r0:r0 + kn, n0:n0 + nn])
                            ot = pio.tile([128, 512], f32, tag="ot")
                            nc.vector.tensor_mul(ot[:kn, :nn], ps[:kn, :nn],
                                                 g2b[:kn, n0:n0 + nn])
                            nc.vector.tensor_add(ot[:kn, :nn], ot[:kn, :nn],
                                                 xg[:kn, :nn])
                            nc.sync.dma_start(out2[r0:r0 + kn, n0:n0 + nn],
                                              ot[:kn, :nn])

    nc.compile()
    return nc


_NC_CACHE = {}


def _get_nc(B, L1, L2, M, H, NC):
    key = (B, L1, L2, M, H, NC)
    if key not in _NC_CACHE:
        _NC_CACHE[key] = build_nc(B, L1, L2, M, H, NC)
    return _NC_CACHE[key]


def make_in_maps(x1, x2, Wq, bq, Wk, bk, Wv1, bv1, Wv2, bv2,
                 Wo1, bo1, Wo2, bo2, gamma1, gamma2, NC):
    B, L1, M = x1.shape
    L2 = x2.shape[1]
    H = M // D
    LSH = L1 // NC
    SCALE = float(D) ** -0.5
    f = np.float32

    def c(a):
        return np.ascontiguousarray(a, dtype=f)

    shared = {
        "x2n": c(np.asarray(x2).reshape(B * L2, M)),
        "Wq": c(Wq), "Wk": c(Wk), "Wv1": c(Wv1), "Wv2": c(Wv2),
        "Wo1": c(Wo1), "Wo2": c(Wo2),
        "bqh": c((SCALE * np.asarray(bq)).reshape(H, D).T),
        "bkh": c(np.asarray(bk).reshape(H, D).T),
        "bv1r": c(np.asarray(bv1).reshape(1, M)),
        "bv2r": c(np.asarray(bv2).reshape(1, M)),
        "bo1r": c(np.asarray(bo1).reshape(1, M)),
        "bo2r": c(np.asarray(bo2).reshape(1, M)),
        "g1": c(np.asarray(gamma1).reshape(1, M)),
        "g2": c(np.asarray(gamma2).reshape(1, M)),
        "ones_d": np.ones((128, 128), np.float32),
    }
    in_maps = []
    for cc in range(NC):
        m = dict(shared)
        m["x1n"] = c(np.asarray(x1)[:, cc * LSH:(cc + 1) * LSH, :]
                     .reshape(B * LSH, M))
        in_maps.append(m)
    return in_maps


def assemble(results, B, L1, L2, M, NC):
    LSH = L1 // NC
    out1 = np.empty((B, L1, M), np.float32)
    for cc in range(NC):
        r = results[cc]["out1"].reshape(B, LSH, M)
        out1[:, cc * LSH:(cc + 1) * LSH, :] = r
    out2 = results[0]["out2"].reshape(B, L2, M)
    return out1, out2


def kernel(x1, x2, Wq, bq, Wk, bk, Wv1, bv1, Wv2, bv2,
           Wo1, bo1, Wo2, bo2, gamma1, gamma2):
    global LAST_RESULT
    NC = 8
    B, L1, M = x1.shape
    L2 = x2.shape[1]
    H = M // D
    nc = _get_nc(B, L1, L2, M, H, NC)
    in_maps = make_in_maps(x1, x2, Wq, bq, Wk, bk, Wv1, bv1, Wv2, bv2,
                           Wo1, bo1, Wo2, bo2, gamma1, gamma2, NC)
    res = run_bass_kernel_spmd(nc, in_maps, core_ids=list(range(NC)),
                               trace=TRACE)
    LAST_RESULT = res
    return assemble(res.results, B, L1, L2, M, NC)
